# revision 12
# baseline (speedup 1.0000x reference)
"""Trainium2 Bass kernel for nn_Attention_21715354649378.

Reference computation (per batch b of 4):
    qkv = w_qkv @ x        x: [256, 4096(=64x64)]   w_qkv: [384, 256]
    q,k,v: [4 heads, 32, 4096];  q *= 32**-0.5
    sim_h = q_h^T k_h   [4096, 4096];  attn = softmax(sim, axis=-1)
    out_h = attn @ v_h^T    -> [4096, 32]
    out = w_out @ concat_heads + b_out   [256, 4096]

Sharding: 8 cores = 4 batches x 2 query-halves. Each core computes K/V for
its full batch plus attention + output projection for its half of the query
pixels. Outputs are disjoint slices -> no collectives.

Device algorithm per core (keys-in-partition layout; exp uses a fixed offset
c=2 that cancels in the softmax normalization):
    krep_h = repl4(W_k,h) x   [128 = 4 copies of k_h(32d), 4096]  bf16
    qrep_h = repl4(s W_q,h) xq [128, 2048] bf16
    vT in TWO dtypes, per head-major [1(ones) | v(32)] 33-blocks:
      vT8 (e4m3) for fp8 DoubleRow PV pairs, vTb (bf16) for the rest
    flat software pipeline over (h, ci) x key-tile groups with two
    alternating PSUM staging pools (4 + 3 banks):
        simT[kt] = krep_h[band].T @ qrep_h[band]   -> PSUM  (band = kt%4)
        exp is split across three engines per key-tile PAIR:
          ACT: activation Exp -> fp8e4 pair tile [128, 1024]
               -> ONE DoubleRow PV matmul per pair (2 key tiles, 256 cyc)
          DVE/POOL: Schraudolph int16 fast-exp -> bf16 -> per-kt bf16 PV
        pv[row 0] = denominator (ones col first), accumulated over 32 kt
        in one of two [33,512] psum slots (partition offset 0/64 alternating)
    normalize: reciprocal(den row) -> broadcast via rank-1 f32r matmul ->
    tensor_mul into outh;  out = W_o @ outh + b_out -> DMA out
"""

import numpy as np
import ml_dtypes

import concourse.bass as bass
import concourse.mybir as mybir
import concourse.tile as tile
from concourse import bacc
from concourse.bass import ts, ds
from concourse.bass_utils import run_bass_kernel_spmd

HEADS = 4
D = 32
HID = 128
C = 256
N = 4096
NQ = 2048
SCALE = D ** -0.5
NCORES = 8

F32 = mybir.dt.float32
F32R = mybir.dt.float32r
BF16 = mybir.dt.bfloat16
I16 = mybir.dt.int16
U8 = mybir.dt.uint8
E4 = mybir.dt.float8e4
EXP = mybir.ActivationFunctionType.Exp
DR = mybir.MatmulPerfMode.DoubleRow

OFFS = 2.0  # exp offset: probs = exp(sim - OFFS); cancels in normalization
# Schraudolph fast-exp constants: bf16 bits of exp(x) ~= int16(x*a + b)
SCH_A = 184.6650
SCH_B = 16256.0 - 8.0 - OFFS * SCH_A

NKT = N // 128  # 32 key tiles per (h, ci)
NCH = NQ // 512  # 4 query chunks
PVLAG = 5  # PV trails exp by this many pending units
# exp engine budget per (h, ci), in pair units: ACT (fp8 pairs / bf16
# singles) vs DVE (Schraudolph bf16). GPSIMD cannot read PSUM on TRN2, so
# only these two engines can drain the QK staging tiles.
ENG_BUDGET = {"A": 9.0, "D": 7.0}


def build_nc():
    nc = bacc.Bacc("TRN2")

    xb = nc.declare_dram_parameter("xb", [C, N], BF16, isOutput=False)
    xq = nc.declare_dram_parameter("xq", [C, NQ], BF16, isOutput=False)
    wqrT = nc.declare_dram_parameter("wqrT", [C, HEADS * HID], BF16, isOutput=False)
    wkrT = nc.declare_dram_parameter("wkrT", [C, HEADS * HID], BF16, isOutput=False)
    wvT = nc.declare_dram_parameter("wvT", [C, HID], BF16, isOutput=False)
    woT = nc.declare_dram_parameter("woT", [HID, C], F32R, isOutput=False)
    bout = nc.declare_dram_parameter("bout", [C, 1], F32, isOutput=False)
    out = nc.declare_dram_parameter("out", [C, NQ], F32, isOutput=True)

    with tile.TileContext(nc) as tc:
        with (
            nc.allow_low_precision(reason="bf16/fp8 attention core"),
            tc.tile_pool(name="persist", bufs=1) as persist,
            tc.tile_pool(name="wts", bufs=1) as wts,
        ):
            # ---- persistent SBUF tensors ----
            x_sb = [
                [
                    persist.tile([128, N // 4], BF16, tag=f"x{i}{j}", name=f"x{i}{j}")
                    for j in range(4)
                ]
                for i in range(2)
            ]
            xq_sb = [
                [
                    persist.tile([128, NQ // 2], BF16, tag=f"xq{i}{j}", name=f"xq{i}{j}")
                    for j in range(2)
                ]
                for i in range(2)
            ]
            krep = [
                persist.tile([128, N], BF16, tag=f"krep{h}", name=f"krep{h}")
                for h in range(HEADS)
            ]
            qrep = [
                persist.tile([128, NQ], BF16, tag=f"qrep{h}", name=f"qrep{h}")
                for h in range(HEADS)
            ]
            # head-major blocks of [v (32) | 1 | pad]; vT8 blocks are
            # padded to 128 cols (dual-fp8 ldweights wants full-width tiles)
            vT8 = persist.tile([128, HEADS * NKT * 128], E4, tag="vT8")
            vTb = persist.tile([128, HEADS * NKT * 33], BF16, tag="vTb")

            wqr_sb = [
                wts.tile([128, HEADS * HID], BF16, tag=f"wqr{i}", name=f"wqr{i}")
                for i in range(2)
            ]
            wkr_sb = [
                wts.tile([128, HEADS * HID], BF16, tag=f"wkr{i}", name=f"wkr{i}")
                for i in range(2)
            ]
            wv_sb = [
                wts.tile([128, HID], BF16, tag=f"wv{i}", name=f"wv{i}")
                for i in range(2)
            ]
            wo_sb = wts.tile([HID, C], F32R, tag="wo")
            bo_sb = [
                wts.tile([128, 1], F32, tag=f"bo{i}", name=f"bo{i}")
                for i in range(2)
            ]
            ones_sb = wts.tile([1, D + 1], F32, tag="ones")
            bias_sb = wts.tile([128, 1], F32, tag="bias")

            # ---- DMA inputs, ordered by first use ----
            for i in range(2):
                nc.sync.dma_start(out=wkr_sb[i][:], in_=wkrT[ds(i * 128, 128), :])
            for i in range(2):
                nc.sync.dma_start(
                    out=x_sb[i][0][:], in_=xb[ds(i * 128, 128), ts(0, N // 4)]
                )
            for i in range(2):
                nc.sync.dma_start(out=wv_sb[i][:], in_=wvT[ds(i * 128, 128), :])
                nc.sync.dma_start(out=wqr_sb[i][:], in_=wqrT[ds(i * 128, 128), :])
            for i in range(2):
                nc.sync.dma_start(
                    out=xq_sb[i][0][:], in_=xq[ds(i * 128, 128), ts(0, NQ // 2)]
                )
            for j in range(1, 4):
                for i in range(2):
                    nc.sync.dma_start(
                        out=x_sb[i][j][:],
                        in_=xb[ds(i * 128, 128), ts(j, N // 4)],
                    )
            for i in range(2):
                nc.sync.dma_start(
                    out=xq_sb[i][1][:], in_=xq[ds(i * 128, 128), ts(1, NQ // 2)]
                )
                nc.sync.dma_start(out=bo_sb[i][:], in_=bout[ds(i * 128, 128), :])
            nc.sync.dma_start(out=wo_sb[:], in_=woT[:, :])
            # vT8 pad cols must be zero; ones col (32) = e4m3 1.0 = 0x38.
            # All on the otherwise-idle GPSIMD, split per head for finer
            # dependency granularity.
            for h in range(HEADS):
                blk = vT8[:, ds(h * NKT * 128, NKT * 128)].bitcast(U8)
                nc.gpsimd.memset(blk, 0)
                nc.gpsimd.memset(
                    blk.rearrange("p (b m) -> p b m", m=128)[:, :, 32:33], 56
                )
            nc.vector.memset(vTb[:], 1.0)
            nc.vector.memset(ones_sb[:], 1.0)
            nc.vector.memset(bias_sb[:], -OFFS)

            with (
                tc.tile_pool(name="qkA", bufs=1, space="PSUM") as qkA,
                tc.tile_pool(name="qkB", bufs=1, space="PSUM") as qkB,
                tc.tile_pool(name="pvp", bufs=1, space="PSUM") as pvp,
                tc.tile_pool(name="pr8", bufs=7) as pr8_pool,
                tc.tile_pool(name="prb", bufs=9) as prb_pool,
                tc.tile_pool(name="norm", bufs=3) as norm_pool,
                tc.tile_pool(name="osb", bufs=2) as osb,
            ):
                _ptog = [0]

                def x_ap(ct, c0, length):
                    t_idx = c0 // (N // 4)
                    return x_sb[ct][t_idx][:, ds(c0 % (N // 4), length)]

                def xq_ap(ct, c0, length):
                    t_idx = c0 // (NQ // 2)
                    return xq_sb[ct][t_idx][:, ds(c0 % (NQ // 2), length)]

                def next_pool():
                    pool = qkA if _ptog[0] == 0 else qkB
                    _ptog[0] ^= 1
                    return pool

                def proj_tile(cols):
                    pool = next_pool()
                    t = pool.tile(
                        [128, (4 if pool is qkA else 3) * 512],
                        F32,
                        tag="qk",
                        name="ps",
                    )
                    return t[:, 0:cols]

                # psum-drain copy engine rotation (ACT-heavy: DVE is the
                # busier engine; GPSIMD cannot read PSUM at all)
                _ceng = [0]

                def copy_rot(dst, src):
                    e = _ceng[0] % 3
                    _ceng[0] += 1
                    if e == 1:
                        nc.vector.tensor_copy(dst, src)
                    else:
                        nc.scalar.copy(dst, src)

                def emit_vt4(kt0):
                    # four key tiles' vT in one staging slot
                    ps = proj_tile(4 * HID)
                    for t in range(4):
                        for ct in range(2):
                            nc.tensor.matmul(
                                ps[:, ts(t, HID)],
                                x_ap(ct, (kt0 + t) * 128, 128),
                                wv_sb[ct][:],
                                start=(ct == 0),
                                stop=(ct == 1),
                            )
                    # ps layout: [128 keys, 4 kt, 4 heads, 32]
                    src = ps.rearrange("p (t w) -> p t w", t=4).rearrange(
                        "p t (h w) -> p t h w", w=32
                    )
                    for h in range(HEADS):
                        dst8 = vT8[
                            :, ds((h * NKT + kt0) * 128, 4 * 128)
                        ].rearrange("p (t w) -> p t w", t=4)[:, :, 0:32]
                        copy_rot(dst8, src[:, :, h, :])
                        dstb = vTb[
                            :, ds(h * NKT * 33 + kt0 * 33, 4 * 33)
                        ].rearrange("p (t w) -> p t w", t=4)[:, :, 0:32]
                        copy_rot(dstb, src[:, :, h, :])

                def emit_k(h, j):
                    ps = proj_tile(512)
                    for ct in range(2):
                        nc.tensor.matmul(
                            ps[:],
                            wkr_sb[ct][:, ts(h, HID)],
                            x_ap(ct, j * 512, 512),
                            start=(ct == 0),
                            stop=(ct == 1),
                        )
                    copy_rot(krep[h][:, ts(j, 512)], ps[:])

                def emit_q(h, j):
                    ps = proj_tile(512)
                    for ct in range(2):
                        nc.tensor.matmul(
                            ps[:],
                            wqr_sb[ct][:, ts(h, HID)],
                            xq_ap(ct, j * 512, 512),
                            start=(ct == 0),
                            stop=(ct == 1),
                        )
                    copy_rot(qrep[h][:, ts(j, 512)], ps[:])

                outh = [
                    osb.tile([HID, 512], F32R, tag=f"outh{c}", name=f"outh{c}")
                    for c in range(NCH)
                ]

                def emit_norm(h, ci, pv):
                    # pv rows 0..31 = out rows, row 32 = denominator
                    den = norm_pool.tile([1, 512], F32, tag="den", name="den")
                    nc.vector.tensor_copy(den[:], pv[32:33, :])
                    rec = norm_pool.tile([1, 512], F32, tag="rec", name="rec")
                    nc.vector.reciprocal_approx_fast(rec[:], den[:])
                    # broadcast 1/den to all partitions on the (otherwise
                    # idle) GPSIMD engine -- SBUF to SBUF only
                    bc = norm_pool.tile([128, 512], F32, tag="bc", name="bc")
                    nc.gpsimd.partition_broadcast(bc[:], rec[:])
                    nc.vector.tensor_mul(
                        outh[ci][ds(32 * h, 32), :],
                        pv[0:32, :],
                        bc[0:32, :],
                    )

                pending = []
                deferred_op = []

                def emit_outproj(ci):
                    for ot in range(2):
                        op = proj_tile(512)
                        nc.tensor.matmul(
                            op,
                            wo_sb[:, ts(ot, 128)],
                            outh[ci][:],
                            start=True,
                            stop=True,
                        )
                        ob = osb.tile([128, 512], F32, tag="ob", name="ob")
                        nc.vector.tensor_scalar_add(ob[:], op, bo_sb[ot][:])
                        nc.sync.dma_start(
                            out=out[ds(ot * 128, 128), ts(ci, 512)], in_=ob[:]
                        )

                def pop_pv(h, ci, pv):
                    kind, tileap, kt0, nkt = pending.pop(0)
                    if kind == "8":
                        lhsT = vT8[
                            :, ds((h * NKT + kt0) * 128, 256)
                        ].rearrange("p (two m) -> p two m", two=2)
                        rhs = tileap.rearrange("p (two n) -> p two n", two=2)
                        nc.tensor.matmul(
                            pv[:, :],
                            lhsT,
                            rhs,
                            start=(kt0 == 0),
                            stop=(kt0 + 2 == NKT),
                            perf_mode=DR,
                            skip_group_check=True,
                        )
                    else:
                        for j in range(nkt):
                            nc.tensor.matmul(
                                pv[0:33, :],
                                vTb[:, ds(h * NKT * 33 + (kt0 + j) * 33, 33)],
                                tileap[:, ts(j, 512)],
                                start=(kt0 + j == 0),
                                stop=(kt0 + j == NKT - 1),
                                skip_group_check=True,
                            )
                    if kt0 + (2 if kind == "8" else nkt) == NKT:
                        emit_norm(h, ci, pv)
                        if h == HEADS - 1:
                            deferred_op.append(ci)

                # per-(h,ci) exp engine assignment state
                def new_budget():
                    return dict(ENG_BUDGET)

                def emit_exp(qk_ap, kts, h, ci, budget):
                    """qk_ap: psum AP [128, len(kts)*512] for consecutive kts.
                    Walks kts; emits pair/single exp ops with engine split."""
                    i = 0
                    nkts = len(kts)
                    while i < nkts:
                        kt = kts[i]
                        if kt % 2 == 0 and i + 1 < nkts:
                            # full pair inside this group
                            if budget["A"] >= 1.0:
                                budget["A"] -= 1.0
                                p8 = pr8_pool.tile(
                                    [128, 1024], E4, tag="p8", name="p8"
                                )
                                nc.scalar.activation(
                                    p8[:],
                                    qk_ap[:, ds(i * 512, 1024)],
                                    EXP,
                                    bias=bias_sb[:],
                                )
                                pending.append(("8", p8, kt, 2))
                            else:
                                budget["D"] -= 1.0
                                pri = prb_pool.tile(
                                    [128, 1024], I16, tag="pb", name="pri"
                                )
                                nc.vector.tensor_scalar(
                                    pri[:],
                                    qk_ap[:, ds(i * 512, 1024)],
                                    SCH_A,
                                    SCH_B,
                                    mybir.AluOpType.mult,
                                    mybir.AluOpType.add,
                                )
                                pending.append(("b", pri.bitcast(BF16), kt, 2))
                            i += 2
                        else:
                            # straddler half -> bf16 single on DVE (or ACT
                            # exp->bf16 when DVE budget is spent)
                            if budget["D"] >= 0.5:
                                budget["D"] -= 0.5
                                pri = prb_pool.tile(
                                    [128, 512], I16, tag="pb", name="prs"
                                )
                                nc.vector.tensor_scalar(
                                    pri[:],
                                    qk_ap[:, ds(i * 512, 512)],
                                    SCH_A,
                                    SCH_B,
                                    mybir.AluOpType.mult,
                                    mybir.AluOpType.add,
                                )
                                pending.append(("b", pri.bitcast(BF16), kt, 1))
                            else:
                                budget["A"] -= 0.5
                                pb = prb_pool.tile(
                                    [128, 512], BF16, tag="pb", name="pbs"
                                )
                                nc.scalar.activation(
                                    pb[:],
                                    qk_ap[:, ds(i * 512, 512)],
                                    EXP,
                                    bias=bias_sb[:],
                                )
                                pending.append(("b", pb, kt, 1))
                            i += 1

                # prologue: first projections
                emit_k(0, 0)
                emit_k(0, 1)
                emit_vt4(0)
                emit_q(0, 0)

                for h in range(HEADS):
                    for ci in range(NCH):
                        pv = pvp.tile([128, 512], F32, tag="pv", name="pv")
                        budget = new_budget()
                        kt = 0
                        g = -2
                        while kt < NKT:
                            g += 2
                            # batch two QK groups back-to-back (halves PE
                            # full<->tiled mode switches)
                            qks = []
                            for _ in range(2):
                                if kt >= NKT:
                                    break
                                pool = next_pool()
                                gsz = min(4 if pool is qkA else 3, NKT - kt)
                                qk = pool.tile(
                                    [128, gsz * 512], F32, tag="qk", name="qkg"
                                )
                                for j in range(gsz):
                                    band = (kt + j) % 4
                                    nc.tensor.matmul(
                                        qk[:, ts(j, 512)],
                                        krep[h][ds(32 * band, 32), ts(kt + j, 128)],
                                        qrep[h][ds(32 * band, 32), ts(ci, 512)],
                                        start=True,
                                        stop=True,
                                        tile_position=(32 * band, 0),
                                    )
                                qks.append((qk, kt, gsz))
                                kt += gsz
                            for qk, kt0, gsz in qks:
                                emit_exp(
                                    qk, list(range(kt0, kt0 + gsz)), h, ci, budget
                                )
                            while len(pending) > PVLAG:
                                pop_pv(h, ci, pv)
                            if g == 4 and deferred_op:
                                emit_outproj(deferred_op.pop(0))
                            # feed upcoming projections into PE idle slots
                            for gg in (g, g + 1):
                                if ci == 0 and h == 0 and gg < 7:
                                    if gg < 6:
                                        emit_k(h, gg + 2)
                                    if 4 * gg + 4 < NKT:
                                        emit_vt4(4 * gg + 4)
                                if ci == 0 and h > 0 and 2 <= gg < 4:
                                    emit_k(h, gg + 4)
                                if gg == 1 and ci < NCH - 1:
                                    emit_q(h, ci + 1)
                                if ci == NCH - 1 and h < HEADS - 1 and 2 <= gg < 9:
                                    if gg == 2:
                                        emit_q(h + 1, 0)
                                    else:
                                        emit_k(h + 1, gg - 3)
                        while pending:
                            pop_pv(h, ci, pv)
                while deferred_op:
                    emit_outproj(deferred_op.pop(0))

    nc.finalize()
    return nc


_NC_CACHE = None


def make_in_maps(x, w_qkv, w_out, b_out):
    bf16 = ml_dtypes.bfloat16
    x = np.ascontiguousarray(np.asarray(x, dtype=np.float32)).reshape(4, C, N)
    w_qkv = np.asarray(w_qkv, dtype=np.float32)
    w_out = np.asarray(w_out, dtype=np.float32)
    b_out = np.asarray(b_out, dtype=np.float32)

    wqT = (w_qkv[0:HID] * SCALE).T                              # [256, 128]
    wkT = w_qkv[HID:2 * HID].T                                  # [256, 128]
    # per-head projection weights, head block replicated 4x along columns
    wqrT = np.ascontiguousarray(
        np.concatenate(
            [np.tile(wqT[:, 32 * h:32 * (h + 1)], (1, 4)) for h in range(HEADS)],
            axis=1,
        )
    ).astype(bf16)
    wkrT = np.ascontiguousarray(
        np.concatenate(
            [np.tile(wkT[:, 32 * h:32 * (h + 1)], (1, 4)) for h in range(HEADS)],
            axis=1,
        )
    ).astype(bf16)
    wvT = np.ascontiguousarray(w_qkv[2 * HID:3 * HID].T).astype(bf16)
    woT = np.ascontiguousarray(w_out.T)                         # [128, 256]
    boutc = np.ascontiguousarray(b_out.reshape(C, 1))
    xbf = x.astype(bf16)

    in_maps = []
    for core in range(NCORES):
        b, half = divmod(core, 2)
        in_maps.append(
            {
                "xb": xbf[b],
                "xq": np.ascontiguousarray(xbf[b][:, half * NQ:(half + 1) * NQ]),
                "wqrT": wqrT,
                "wkrT": wkrT,
                "wvT": wvT,
                "woT": woT,
                "bout": boutc,
            }
        )
    return in_maps


def kernel(x, w_qkv, w_out, b_out):
    global _NC_CACHE
    if _NC_CACHE is None:
        _NC_CACHE = build_nc()
    nc = _NC_CACHE
    in_maps = make_in_maps(x, w_qkv, w_out, b_out)
    res = run_bass_kernel_spmd(nc, in_maps, core_ids=list(range(NCORES)))
    out = np.empty((4, C, N), dtype=np.float32)
    for core in range(NCORES):
        b, half = divmod(core, 2)
        out[b][:, half * NQ:(half + 1) * NQ] = res.results[core]["out"]
    return out.reshape(4, C, 64, 64)


# revision 14
# speedup vs baseline: 1.1476x; 1.1476x over previous
"""Trainium2 Bass kernel for nn_Attention_21715354649378.

Reference computation (per batch b of 4):
    qkv = w_qkv @ x        x: [256, 4096(=64x64)]   w_qkv: [384, 256]
    q,k,v: [4 heads, 32, 4096];  q *= 32**-0.5
    sim_h = q_h^T k_h   [4096, 4096];  attn = softmax(sim, axis=-1)
    out_h = attn @ v_h^T    -> [4096, 32]
    out = w_out @ concat_heads + b_out   [256, 4096]

Sharding: 8 cores = 4 batches x 2 query-halves. Each core computes K/V for
its full batch plus attention + output projection for its half of the query
pixels. Outputs are disjoint slices -> no collectives.

Device algorithm per core (keys-in-partition layout; exp uses a fixed offset
c=2 that cancels in the softmax normalization):
    krep_h = repl4(W_k,h) x   [128 = 4 copies of k_h(32d), 4096]  bf16
    qrep_h = repl4(s W_q,h) xq [128, 2048] bf16
    vT in TWO dtypes, per head-major [1(ones) | v(32)] 33-blocks:
      vT8 (e4m3) for fp8 DoubleRow PV pairs, vTb (bf16) for the rest
    flat software pipeline over (h, ci) x key-tile groups with two
    alternating PSUM staging pools (4 + 3 banks):
        simT[kt] = krep_h[band].T @ qrep_h[band]   -> PSUM  (band = kt%4)
        exp is split across three engines per key-tile PAIR:
          ACT: activation Exp -> fp8e4 pair tile [128, 1024]
               -> ONE DoubleRow PV matmul per pair (2 key tiles, 256 cyc)
          DVE/POOL: Schraudolph int16 fast-exp -> bf16 -> per-kt bf16 PV
        pv[row 0] = denominator (ones col first), accumulated over 32 kt
        in one of two [33,512] psum slots (partition offset 0/64 alternating)
    normalize: reciprocal(den row) -> broadcast via rank-1 f32r matmul ->
    tensor_mul into outh;  out = W_o @ outh + b_out -> DMA out
"""

import numpy as np
import ml_dtypes

import concourse.bass as bass
import concourse.mybir as mybir
import concourse.tile as tile
from concourse import bacc
from concourse.bass import ts, ds
from concourse.bass_utils import run_bass_kernel_spmd

HEADS = 4
D = 32
HID = 128
C = 256
N = 4096
NQ = 2048
SCALE = D ** -0.5
NCORES = 8

F32 = mybir.dt.float32
F32R = mybir.dt.float32r
BF16 = mybir.dt.bfloat16
I16 = mybir.dt.int16
U8 = mybir.dt.uint8
E4 = mybir.dt.float8e4
EXP = mybir.ActivationFunctionType.Exp
DR = mybir.MatmulPerfMode.DoubleRow

OFFS = 2.0  # exp offset: probs = exp(sim - OFFS); cancels in normalization
# Schraudolph fast-exp constants: bf16 bits of exp(x) ~= int16(x*a + b)
SCH_A = 184.6650
SCH_B = 16256.0 - 8.0 - OFFS * SCH_A

NKT = N // 128  # 32 key tiles per (h, ci)
NCH = NQ // 512  # 4 query chunks
PVLAG = 6  # PV trails exp by this many pending units
# exp engine budget per (h, ci), in KEY TILES: ACT (activation -> fp8e4,
# DoubleRow pairs + plain-fp8 singles) vs DVE (Schraudolph -> bf16).
# GPSIMD cannot read PSUM on TRN2, so only these two engines can drain the
# QK staging tiles. Each staging group goes WHOLLY to one engine (one exp
# instruction per group) and groups alternate by remaining budget.
ENG_BUDGET = {"A": 18.0, "D": 14.0}


def build_nc():
    nc = bacc.Bacc("TRN2")

    xb = nc.declare_dram_parameter("xb", [C, N], BF16, isOutput=False)
    xq = nc.declare_dram_parameter("xq", [C, NQ], BF16, isOutput=False)
    wqrT = nc.declare_dram_parameter("wqrT", [C, HEADS * HID], BF16, isOutput=False)
    wkrT = nc.declare_dram_parameter("wkrT", [C, HEADS * HID], BF16, isOutput=False)
    wvT = nc.declare_dram_parameter("wvT", [C, HID], BF16, isOutput=False)
    woT = nc.declare_dram_parameter("woT", [HID, C], F32R, isOutput=False)
    bout = nc.declare_dram_parameter("bout", [C, 1], F32, isOutput=False)
    out = nc.declare_dram_parameter("out", [C, NQ], F32, isOutput=True)

    with tile.TileContext(nc) as tc:
        with (
            nc.allow_low_precision(reason="bf16/fp8 attention core"),
            tc.tile_pool(name="persist", bufs=1) as persist,
            tc.tile_pool(name="wts", bufs=1) as wts,
        ):
            # ---- persistent SBUF tensors ----
            x_sb = [
                [
                    persist.tile([128, N // 4], BF16, tag=f"x{i}{j}", name=f"x{i}{j}")
                    for j in range(4)
                ]
                for i in range(2)
            ]
            xq_sb = [
                [
                    persist.tile([128, NQ // 2], BF16, tag=f"xq{i}{j}", name=f"xq{i}{j}")
                    for j in range(2)
                ]
                for i in range(2)
            ]
            krep = [
                persist.tile([128, N], BF16, tag=f"krep{h}", name=f"krep{h}")
                for h in range(HEADS)
            ]
            qrep = [
                persist.tile([128, NQ], BF16, tag=f"qrep{h}", name=f"qrep{h}")
                for h in range(HEADS)
            ]
            # head-major blocks of [v (32) | 1 | pad]; vT8 blocks are
            # padded to 128 cols (dual-fp8 ldweights wants full-width tiles)
            vT8 = persist.tile([128, HEADS * NKT * 128], E4, tag="vT8")
            vTb = persist.tile([128, HEADS * NKT * 33], BF16, tag="vTb")

            wqr_sb = [
                wts.tile([128, HEADS * HID], BF16, tag=f"wqr{i}", name=f"wqr{i}")
                for i in range(2)
            ]
            wkr_sb = [
                wts.tile([128, HEADS * HID], BF16, tag=f"wkr{i}", name=f"wkr{i}")
                for i in range(2)
            ]
            wv_sb = [
                wts.tile([128, HID], BF16, tag=f"wv{i}", name=f"wv{i}")
                for i in range(2)
            ]
            wo_sb = wts.tile([HID, C], F32R, tag="wo")
            bo_sb = [
                wts.tile([128, 1], F32, tag=f"bo{i}", name=f"bo{i}")
                for i in range(2)
            ]
            ones_sb = wts.tile([1, D + 1], F32, tag="ones")
            bias_sb = wts.tile([128, 1], F32, tag="bias")

            # ---- DMA inputs, ordered by first use ----
            for i in range(2):
                nc.sync.dma_start(out=wkr_sb[i][:], in_=wkrT[ds(i * 128, 128), :])
            for i in range(2):
                nc.sync.dma_start(
                    out=x_sb[i][0][:], in_=xb[ds(i * 128, 128), ts(0, N // 4)]
                )
            for i in range(2):
                nc.sync.dma_start(out=wv_sb[i][:], in_=wvT[ds(i * 128, 128), :])
                nc.sync.dma_start(out=wqr_sb[i][:], in_=wqrT[ds(i * 128, 128), :])
            for i in range(2):
                nc.sync.dma_start(
                    out=xq_sb[i][0][:], in_=xq[ds(i * 128, 128), ts(0, NQ // 2)]
                )
            for j in range(1, 4):
                for i in range(2):
                    nc.sync.dma_start(
                        out=x_sb[i][j][:],
                        in_=xb[ds(i * 128, 128), ts(j, N // 4)],
                    )
            for i in range(2):
                nc.sync.dma_start(
                    out=xq_sb[i][1][:], in_=xq[ds(i * 128, 128), ts(1, NQ // 2)]
                )
                nc.sync.dma_start(out=bo_sb[i][:], in_=bout[ds(i * 128, 128), :])
            nc.sync.dma_start(out=wo_sb[:], in_=woT[:, :])
            # vT8 pad cols must be zero; ones col (32) = e4m3 1.0 = 0x38.
            # All on the otherwise-idle GPSIMD, split per head for finer
            # dependency granularity.
            for h in range(HEADS):
                blk = vT8[:, ds(h * NKT * 128, NKT * 128)].bitcast(U8)
                nc.gpsimd.memset(blk, 0)
                nc.gpsimd.memset(
                    blk.rearrange("p (b m) -> p b m", m=128)[:, :, 32:33], 56
                )
            nc.vector.memset(vTb[:], 1.0)
            nc.vector.memset(ones_sb[:], 1.0)
            nc.vector.memset(bias_sb[:], -OFFS)

            with (
                tc.tile_pool(name="qkA", bufs=1, space="PSUM") as qkA,
                tc.tile_pool(name="qkB", bufs=1, space="PSUM") as qkB,
                tc.tile_pool(name="pvp", bufs=1, space="PSUM") as pvp,
                tc.tile_pool(name="pr8", bufs=7) as pr8_pool,
                tc.tile_pool(name="prb", bufs=9) as prb_pool,
                tc.tile_pool(name="norm", bufs=3) as norm_pool,
                tc.tile_pool(name="osb", bufs=2) as osb,
            ):
                _ptog = [0]

                def x_ap(ct, c0, length):
                    t_idx = c0 // (N // 4)
                    return x_sb[ct][t_idx][:, ds(c0 % (N // 4), length)]

                def xq_ap(ct, c0, length):
                    t_idx = c0 // (NQ // 2)
                    return xq_sb[ct][t_idx][:, ds(c0 % (NQ // 2), length)]

                def next_pool():
                    pool = qkA if _ptog[0] == 0 else qkB
                    _ptog[0] ^= 1
                    return pool

                def proj_tile(cols):
                    pool = next_pool()
                    t = pool.tile(
                        [128, (4 if pool is qkA else 3) * 512],
                        F32,
                        tag="qk",
                        name="ps",
                    )
                    return t[:, 0:cols]

                # psum-drain copy engine rotation (ACT-heavy: DVE is the
                # busier engine; GPSIMD cannot read PSUM at all)
                _ceng = [0]

                def copy_rot(dst, src):
                    e = _ceng[0] % 3
                    _ceng[0] += 1
                    if e == 1:
                        nc.vector.tensor_copy(dst, src)
                    else:
                        nc.scalar.copy(dst, src)

                def emit_vt4(kt0):
                    # four key tiles' vT in one staging slot
                    ps = proj_tile(4 * HID)
                    for t in range(4):
                        for ct in range(2):
                            nc.tensor.matmul(
                                ps[:, ts(t, HID)],
                                x_ap(ct, (kt0 + t) * 128, 128),
                                wv_sb[ct][:],
                                start=(ct == 0),
                                stop=(ct == 1),
                            )
                    # ps layout: [128 keys, 4 kt, 4 heads, 32]
                    src = ps.rearrange("p (t w) -> p t w", t=4).rearrange(
                        "p t (h w) -> p t h w", w=32
                    )
                    for h in range(HEADS):
                        dst8 = vT8[
                            :, ds((h * NKT + kt0) * 128, 4 * 128)
                        ].rearrange("p (t w) -> p t w", t=4)[:, :, 0:32]
                        copy_rot(dst8, src[:, :, h, :])
                        dstb = vTb[
                            :, ds(h * NKT * 33 + kt0 * 33, 4 * 33)
                        ].rearrange("p (t w) -> p t w", t=4)[:, :, 0:32]
                        copy_rot(dstb, src[:, :, h, :])

                def emit_k(h, j):
                    ps = proj_tile(512)
                    for ct in range(2):
                        nc.tensor.matmul(
                            ps[:],
                            wkr_sb[ct][:, ts(h, HID)],
                            x_ap(ct, j * 512, 512),
                            start=(ct == 0),
                            stop=(ct == 1),
                        )
                    copy_rot(krep[h][:, ts(j, 512)], ps[:])

                def emit_q(h, j):
                    ps = proj_tile(512)
                    for ct in range(2):
                        nc.tensor.matmul(
                            ps[:],
                            wqr_sb[ct][:, ts(h, HID)],
                            xq_ap(ct, j * 512, 512),
                            start=(ct == 0),
                            stop=(ct == 1),
                        )
                    copy_rot(qrep[h][:, ts(j, 512)], ps[:])

                outh = [
                    osb.tile([HID, 512], F32R, tag=f"outh{c}", name=f"outh{c}")
                    for c in range(NCH)
                ]

                def emit_norm(h, ci, pv):
                    # pv rows 0..31 = out rows, row 32 = denominator
                    den = norm_pool.tile([1, 512], F32, tag="den", name="den")
                    nc.vector.tensor_copy(den[:], pv[32:33, :])
                    rec = norm_pool.tile([1, 512], F32, tag="rec", name="rec")
                    nc.vector.reciprocal_approx_fast(rec[:], den[:])
                    # broadcast 1/den to all partitions on the (otherwise
                    # idle) GPSIMD engine -- SBUF to SBUF only
                    bc = norm_pool.tile([128, 512], F32, tag="bc", name="bc")
                    nc.gpsimd.partition_broadcast(bc[:], rec[:])
                    nc.vector.tensor_mul(
                        outh[ci][ds(32 * h, 32), :],
                        pv[0:32, :],
                        bc[0:32, :],
                    )

                pending = []
                deferred_op = []

                def emit_outproj(ci):
                    for ot in range(2):
                        op = proj_tile(512)
                        nc.tensor.matmul(
                            op,
                            wo_sb[:, ts(ot, 128)],
                            outh[ci][:],
                            start=True,
                            stop=True,
                        )
                        ob = osb.tile([128, 512], F32, tag="ob", name="ob")
                        nc.vector.tensor_scalar_add(ob[:], op, bo_sb[ot][:])
                        nc.sync.dma_start(
                            out=out[ds(ot * 128, 128), ts(ci, 512)], in_=ob[:]
                        )

                def pop_pv(h, ci, pv):
                    kind, tileap, kt0, nkt = pending.pop(0)
                    if kind == "8":
                        lhsT = vT8[
                            :, ds((h * NKT + kt0) * 128, 256)
                        ].rearrange("p (two m) -> p two m", two=2)
                        rhs = tileap.rearrange("p (two n) -> p two n", two=2)
                        nc.tensor.matmul(
                            pv[:, :],
                            lhsT,
                            rhs,
                            start=(kt0 == 0),
                            stop=(kt0 + 2 == NKT),
                            perf_mode=DR,
                            skip_group_check=True,
                        )
                    elif kind == "s8":
                        nc.tensor.matmul(
                            pv[:, :],
                            vT8[:, ds((h * NKT + kt0) * 128, 128)],
                            tileap,
                            start=(kt0 == 0),
                            stop=(kt0 + 1 == NKT),
                            skip_group_check=True,
                        )
                    else:
                        for j in range(nkt):
                            nc.tensor.matmul(
                                pv[0:33, :],
                                vTb[:, ds(h * NKT * 33 + (kt0 + j) * 33, 33)],
                                tileap[:, ts(j, 512)],
                                start=(kt0 + j == 0),
                                stop=(kt0 + j == NKT - 1),
                                skip_group_check=True,
                            )
                    if kt0 + nkt == NKT:
                        emit_norm(h, ci, pv)
                        if h == HEADS - 1:
                            deferred_op.append(ci)

                # per-(h,ci) exp engine assignment state
                def new_budget():
                    return dict(ENG_BUDGET)

                def emit_exp(qk_ap, kts, h, ci, budget):
                    """qk_ap: psum AP [128, len(kts)*512] for consecutive
                    kts. One exp instruction for the whole group, on the
                    engine with more remaining budget (keeps ACT and DVE
                    working in parallel across alternating groups)."""
                    gsz = len(kts)
                    use_a = budget["A"] >= budget["D"]
                    budget["A" if use_a else "D"] -= gsz
                    if use_a:
                        p8 = pr8_pool.tile(
                            [128, gsz * 512], E4, tag="p8", name="p8"
                        )
                        nc.scalar.activation(
                            p8[:], qk_ap[:, ds(0, gsz * 512)], EXP,
                            bias=bias_sb[:],
                        )
                        i = 0
                        while i < gsz:
                            kt = kts[i]
                            if kt % 2 == 0 and i + 1 < gsz:
                                pending.append(("8", p8[:, ds(i * 512, 1024)], kt, 2))
                                i += 2
                            else:
                                pending.append(("s8", p8[:, ds(i * 512, 512)], kt, 1))
                                i += 1
                    else:
                        pri = prb_pool.tile(
                            [128, gsz * 512], I16, tag="pb", name="pri"
                        )
                        nc.vector.tensor_scalar(
                            pri[:], qk_ap[:, ds(0, gsz * 512)],
                            SCH_A, SCH_B,
                            mybir.AluOpType.mult, mybir.AluOpType.add,
                        )
                        pending.append(("b", pri.bitcast(BF16), kts[0], gsz))

                # prologue: first projections
                emit_k(0, 0)
                emit_k(0, 1)
                emit_vt4(0)
                emit_q(0, 0)

                for h in range(HEADS):
                    for ci in range(NCH):
                        pv = pvp.tile([128, 512], F32, tag="pv", name="pv")
                        budget = new_budget()
                        kt = 0
                        g = -2
                        while kt < NKT:
                            g += 2
                            # batch two QK groups back-to-back (halves PE
                            # full<->tiled mode switches)
                            qks = []
                            for _ in range(2):
                                if kt >= NKT:
                                    break
                                pool = next_pool()
                                gsz = min(4 if pool is qkA else 3, NKT - kt)
                                qk = pool.tile(
                                    [128, gsz * 512], F32, tag="qk", name="qkg"
                                )
                                for j in range(gsz):
                                    band = (kt + j) % 4
                                    nc.tensor.matmul(
                                        qk[:, ts(j, 512)],
                                        krep[h][ds(32 * band, 32), ts(kt + j, 128)],
                                        qrep[h][ds(32 * band, 32), ts(ci, 512)],
                                        start=True,
                                        stop=True,
                                        tile_position=(32 * band, 0),
                                    )
                                qks.append((qk, kt, gsz))
                                kt += gsz
                            for qk, kt0, gsz in qks:
                                emit_exp(
                                    qk, list(range(kt0, kt0 + gsz)), h, ci, budget
                                )
                            while len(pending) > PVLAG:
                                pop_pv(h, ci, pv)
                            if g == 4 and deferred_op:
                                emit_outproj(deferred_op.pop(0))
                            # feed upcoming projections into PE idle slots
                            for gg in (g, g + 1):
                                if ci == 0 and h == 0 and gg < 7:
                                    if gg < 6:
                                        emit_k(h, gg + 2)
                                    if 4 * gg + 4 < NKT:
                                        emit_vt4(4 * gg + 4)
                                if ci == 0 and h > 0 and 2 <= gg < 4:
                                    emit_k(h, gg + 4)
                                if gg == 1 and ci < NCH - 1:
                                    emit_q(h, ci + 1)
                                if ci == NCH - 1 and h < HEADS - 1 and 2 <= gg < 9:
                                    if gg == 2:
                                        emit_q(h + 1, 0)
                                    else:
                                        emit_k(h + 1, gg - 3)
                        while pending:
                            pop_pv(h, ci, pv)
                while deferred_op:
                    emit_outproj(deferred_op.pop(0))

    nc.finalize()
    return nc


_NC_CACHE = None


def make_in_maps(x, w_qkv, w_out, b_out):
    bf16 = ml_dtypes.bfloat16
    x = np.ascontiguousarray(np.asarray(x, dtype=np.float32)).reshape(4, C, N)
    w_qkv = np.asarray(w_qkv, dtype=np.float32)
    w_out = np.asarray(w_out, dtype=np.float32)
    b_out = np.asarray(b_out, dtype=np.float32)

    wqT = (w_qkv[0:HID] * SCALE).T                              # [256, 128]
    wkT = w_qkv[HID:2 * HID].T                                  # [256, 128]
    # per-head projection weights, head block replicated 4x along columns
    wqrT = np.ascontiguousarray(
        np.concatenate(
            [np.tile(wqT[:, 32 * h:32 * (h + 1)], (1, 4)) for h in range(HEADS)],
            axis=1,
        )
    ).astype(bf16)
    wkrT = np.ascontiguousarray(
        np.concatenate(
            [np.tile(wkT[:, 32 * h:32 * (h + 1)], (1, 4)) for h in range(HEADS)],
            axis=1,
        )
    ).astype(bf16)
    wvT = np.ascontiguousarray(w_qkv[2 * HID:3 * HID].T).astype(bf16)
    woT = np.ascontiguousarray(w_out.T)                         # [128, 256]
    boutc = np.ascontiguousarray(b_out.reshape(C, 1))
    xbf = x.astype(bf16)

    in_maps = []
    for core in range(NCORES):
        b, half = divmod(core, 2)
        in_maps.append(
            {
                "xb": xbf[b],
                "xq": np.ascontiguousarray(xbf[b][:, half * NQ:(half + 1) * NQ]),
                "wqrT": wqrT,
                "wkrT": wkrT,
                "wvT": wvT,
                "woT": woT,
                "bout": boutc,
            }
        )
    return in_maps


def kernel(x, w_qkv, w_out, b_out):
    global _NC_CACHE
    if _NC_CACHE is None:
        _NC_CACHE = build_nc()
    nc = _NC_CACHE
    in_maps = make_in_maps(x, w_qkv, w_out, b_out)
    res = run_bass_kernel_spmd(nc, in_maps, core_ids=list(range(NCORES)))
    out = np.empty((4, C, N), dtype=np.float32)
    for core in range(NCORES):
        b, half = divmod(core, 2)
        out[b][:, half * NQ:(half + 1) * NQ] = res.results[core]["out"]
    return out.reshape(4, C, 64, 64)
